# revision 2
# baseline (speedup 1.0000x reference)
"""Bahdanau-style attention kernel for Trainium2, SPMD over 8 NeuronCores.

Problem (per reference):
    h     = tanh(enc @ W1 + W1_b + dec @ W2 + W2_b)      # [B, H, W, U]
    score = h @ V + V_b                                   # [B, H, W, 1]
    attn  = softmax(score, axis=H)
    ctx   = sum_{H,W} attn * enc                          # [B, D]
    returns (ctx, attn)

Sharding: data-parallel over batch B=128 -> 16 batches/core; weights replicated.

Per-core dataflow (all matmuls in float32r = fast fp32 path on the PE):
  - X = enc[b] viewed [R=768, D=512] is DMA'd naturally (rows on partitions).
  - X^T obtained via PE transpose-mode (24 [128,128] tiles/batch).
  - H^T[u, r] = sum_d W1[d, u] X^T[d, r] accumulated in PSUM; ACT applies
    tanh with the per-u bias vec2[u] = (dec[b] @ W2 + W1_b + W2_b)[u] fused in.
  - score^T [1, R] = sum_u V[u] H^T[u, r] (M=1 matmuls, PSUM-accumulated).
  - exp on ACT; softmax over the H axis via strided-AP reduce on DVE,
    batched over groups of 8 batches (rows on partitions).
  - ctx[b] = attn_col^T @ X via M=1 matmuls over the 6 row-chunks.

V_b is omitted: softmax over H is invariant to adding a constant scalar, so
V_b affects neither output.
"""

import numpy as np

B, H, W, D, U = 128, 16, 48, 512, 512
R = H * W            # 768 rows per batch (h-major: r = h*48 + w)
NT = R // 128        # 6 row tiles
NCH = D // 128       # 4 contraction chunks
N_CORES = 8
BL = B // N_CORES    # 16 batches per core
G = 8                # softmax/ctx group size (bounds SBUF residency of X)

_cache = {}


def _build():
    import concourse.mybir as mybir
    import concourse.tile as tile
    from concourse import bacc
    from concourse.bass import ts
    from concourse.masks import make_identity

    F32 = mybir.dt.float32
    F32R = mybir.dt.float32r
    AF = mybir.ActivationFunctionType

    nc = bacc.Bacc()
    enc = nc.dram_tensor("enc", [BL, R, D], F32R, kind="ExternalInput")
    dec = nc.dram_tensor("dec", [BL, D], F32, kind="ExternalInput")
    w1 = nc.dram_tensor("w1", [D, U], F32R, kind="ExternalInput")
    w2 = nc.dram_tensor("w2", [D, U], F32, kind="ExternalInput")
    w1b = nc.dram_tensor("w1b", [1, U], F32, kind="ExternalInput")
    w2b = nc.dram_tensor("w2b", [1, U], F32, kind="ExternalInput")
    v = nc.dram_tensor("v", [U, 1], F32R, kind="ExternalInput")
    out_ctx = nc.dram_tensor("out_ctx", [BL, D], F32, kind="ExternalOutput")
    out_attn = nc.dram_tensor("out_attn", [BL, R], F32, kind="ExternalOutput")

    with tile.TileContext(nc) as tc:
        with (
            tc.tile_pool(name="const", bufs=1) as constp,
            tc.tile_pool(name="xpool", bufs=G + 1) as xpool,
            tc.tile_pool(name="xtp", bufs=2) as xtp,
            tc.tile_pool(name="htp", bufs=6) as htp,
            tc.tile_pool(name="grp", bufs=2) as grpp,
            tc.tile_pool(name="psum", bufs=2, space="PSUM") as psump,
        ):
            # ---------------- preamble: constants ----------------
            ident_f = constp.tile([128, 128], F32)
            make_identity(nc, ident_f[:])
            ident = constp.tile([128, 128], F32R)
            nc.vector.tensor_copy(ident[:], ident_f[:])

            w1_sb = constp.tile([128, NCH, U], F32R)
            nc.sync.dma_start(w1_sb[:], w1[:].rearrange("(c p) u -> p c u", p=128))
            v_sb = constp.tile([128, NCH], F32R)
            nc.sync.dma_start(v_sb[:], v[:].rearrange("(c p) o -> p (c o)", p=128))

            # vec2[u, b] = (dec[b] @ W2)[u] + W1_b[u] + W2_b[u]  (plain fp32)
            w2_sb = constp.tile([128, NCH, U], F32)
            nc.sync.dma_start(w2_sb[:], w2[:].rearrange("(c p) u -> p c u", p=128))
            dec_sb = constp.tile([BL, D], F32)
            nc.sync.dma_start(dec_sb[:], dec[:])
            p_dt = psump.tile([128, NCH, BL], F32, tag="pt")
            for c in range(NCH):
                nc.tensor.transpose(
                    p_dt[:, c, :], dec_sb[0:BL, ts(c, 128)], ident_f[0:BL, 0:BL]
                )
            decT = constp.tile([128, NCH, BL], F32)
            nc.vector.tensor_copy(decT[:], p_dt[:])

            b1_sb = constp.tile([1, U], F32)
            nc.sync.dma_start(b1_sb[:], w1b[:])
            b2_sb = constp.tile([1, U], F32)
            nc.sync.dma_start(b2_sb[:], w2b[:])
            bsum = constp.tile([1, U], F32)
            nc.vector.tensor_add(bsum[:], b1_sb[:], b2_sb[:])
            p_bs = psump.tile([128, NCH], F32, tag="pt")
            for c in range(NCH):
                nc.tensor.transpose(
                    p_bs[:, c : c + 1], bsum[0:1, ts(c, 128)], ident_f[0:1, 0:1]
                )
            bsumT = constp.tile([128, NCH], F32)
            nc.vector.tensor_copy(bsumT[:], p_bs[:])

            vec2T = constp.tile([128, NCH, BL], F32)
            for u in range(NCH):
                p_v2 = psump.tile([128, BL], F32, tag="pt")
                for c in range(NCH):
                    nc.tensor.matmul(
                        p_v2[:],
                        w2_sb[:, c, ts(u, 128)],
                        decT[:, c, :],
                        start=(c == 0),
                        stop=(c == NCH - 1),
                    )
                nc.scalar.activation(
                    vec2T[:, u, :], p_v2[:], AF.Identity,
                    bias=bsumT[:, u : u + 1], scale=1.0,
                )

            # ---------------- main loop ----------------
            for g in range(BL // G):
                x_tiles = []
                sexp_g = grpp.tile([G, R], F32, tag="sexp")
                for bb in range(G):
                    b = g * G + bb
                    x_nat = xpool.tile([128, NT, D], F32R, tag="x")
                    nc.sync.dma_start(
                        x_nat[:], enc[b].rearrange("(t p) d -> p t d", p=128)
                    )
                    x_tiles.append(x_nat)

                    # X^T via PE transpose
                    xt = xtp.tile([128, NCH, R], F32R, tag="xt")
                    for c in range(NCH):
                        p_t = psump.tile([128, R], F32R, tag="pt")
                        for t in range(NT):
                            nc.tensor.transpose(
                                p_t[:, ts(t, 128)], x_nat[:, t, ts(c, 128)], ident[:]
                            )
                        nc.vector.tensor_copy(xt[:, c, :], p_t[:])

                    # H^T = tanh(W1^T X^T + vec2)
                    ht_tiles = []
                    for u in range(NCH):
                        p_h = psump.tile([128, R], F32, tag="ph")
                        for hs in (slice(0, 512), slice(512, R)):
                            for c in range(NCH):
                                nc.tensor.matmul(
                                    p_h[:, hs],
                                    w1_sb[:, c, ts(u, 128)],
                                    xt[:, c, hs],
                                    start=(c == 0),
                                    stop=(c == NCH - 1),
                                )
                        ht_u = htp.tile([128, R], F32R, tag="ht")
                        nc.scalar.activation(
                            ht_u[:], p_h[:], AF.Tanh,
                            bias=vec2T[:, u, b : b + 1], scale=1.0,
                        )
                        ht_tiles.append(ht_u)

                    # score^T = V^T H^T  (M=1), then exp
                    p_s = psump.tile([1, R], F32, tag="pt")
                    for hs in (slice(0, 512), slice(512, R)):
                        for u in range(NCH):
                            nc.tensor.matmul(
                                p_s[:, hs],
                                v_sb[:, u : u + 1],
                                ht_tiles[u][:, hs],
                                start=(u == 0),
                                stop=(u == NCH - 1),
                            )
                    se_b = grpp.tile([1, R], F32, tag="se_b", bufs=3)
                    nc.scalar.activation(se_b[:], p_s[:], AF.Exp)
                    nc.sync.dma_start(sexp_g[bb : bb + 1, :], se_b[:])

                # softmax over h (strided segments; h stride is W in memory)
                attn_g = grpp.tile([G, R], F32R, tag="attn")
                ssum = grpp.tile([G, W], F32, tag="ssum")
                sexp_wh = sexp_g[:].rearrange("p (h w) -> p w h", h=H, w=W)
                nc.vector.tensor_reduce(
                    ssum[:], sexp_wh, op=mybir.AluOpType.add, axis=mybir.AxisListType.X
                )
                rsum = grpp.tile([G, W], F32, tag="rsum")
                nc.vector.reciprocal(rsum[:], ssum[:])
                attn_wh = attn_g[:].rearrange("p (h w) -> p w h", h=H, w=W)
                nc.vector.tensor_tensor(
                    attn_wh,
                    sexp_wh,
                    rsum[:].unsqueeze(2).broadcast_to([G, W, H]),
                    op=mybir.AluOpType.mult,
                )
                nc.sync.dma_start(out_attn[g * G : (g + 1) * G, :], attn_g[:].bitcast(F32))

                # attn columns for the context matmuls
                p_at = psump.tile([128, NT, G], F32, tag="pt")
                for t in range(NT):
                    nc.tensor.transpose(
                        p_at[:, t, :], attn_g[0:G, ts(t, 128)].bitcast(F32),
                        ident_f[0:G, 0:G],
                    )
                attT = grpp.tile([128, NT, G], F32R, tag="attT")
                nc.vector.tensor_copy(attT[:], p_at[:])

                # ctx[b] = attn^T X
                for bb in range(G):
                    b = g * G + bb
                    p_c = psump.tile([1, D], F32, tag="ph")
                    for t in range(NT):
                        nc.tensor.matmul(
                            p_c[:],
                            attT[:, t, bb : bb + 1],
                            x_tiles[bb][:, t, :],
                            start=(t == 0),
                            stop=(t == NT - 1),
                        )
                    ctx_b = grpp.tile([1, D], F32, tag="ctx_b", bufs=3)
                    nc.vector.tensor_copy(ctx_b[:], p_c[:])
                    nc.sync.dma_start(out_ctx[b : b + 1, :], ctx_b[:])
    nc.compile()
    return nc


def kernel(dec_hidden, enc_output, W1_w, W1_b, W2_w, W2_b, V_w, V_b):
    from concourse.bass_utils import run_bass_kernel_spmd

    if "nc" not in _cache:
        _cache["nc"] = _build()
    nc = _cache["nc"]

    enc_full = np.ascontiguousarray(np.asarray(enc_output, dtype=np.float32)).reshape(B, R, D)
    dec_full = np.ascontiguousarray(np.asarray(dec_hidden, dtype=np.float32))
    w1 = np.ascontiguousarray(np.asarray(W1_w, dtype=np.float32))
    w2 = np.ascontiguousarray(np.asarray(W2_w, dtype=np.float32))
    w1b = np.ascontiguousarray(np.asarray(W1_b, dtype=np.float32)).reshape(1, U)
    w2b = np.ascontiguousarray(np.asarray(W2_b, dtype=np.float32)).reshape(1, U)
    vw = np.ascontiguousarray(np.asarray(V_w, dtype=np.float32))

    in_maps = []
    for i in range(N_CORES):
        sl = slice(i * BL, (i + 1) * BL)
        in_maps.append(
            {
                "enc": enc_full[sl],
                "dec": dec_full[sl],
                "w1": w1,
                "w2": w2,
                "w1b": w1b,
                "w2b": w2b,
                "v": vw,
            }
        )

    res = run_bass_kernel_spmd(nc, in_maps, core_ids=list(range(N_CORES)))
    ctx = np.concatenate([res.results[i]["out_ctx"] for i in range(N_CORES)], axis=0)
    attn = np.concatenate([res.results[i]["out_attn"] for i in range(N_CORES)], axis=0)
    return ctx, attn.reshape(B, H, W, 1)
